# revision 4
# baseline (speedup 1.0000x reference)
"""Embedding-lookup + row-wise dot kernel for Trainium2 (8 NeuronCores).

Problem (hardcoded, self-contained):
    users:       [16384] int   (values < 1_000_000)
    movies:      [16384] int   (values < 100_000)
    user_table:  [1_000_000, 64] f32
    movie_table: [100_000, 64] f32
    out = sum(user_table[users] * movie_table[movies], axis=-1, keepdims=True)

Strategy (v2 — 4-queue SWDGE gather via InstDMAGatherAnt):
  The SWDGE Q7 descriptor-generation rate is ~100 descs/us per queue pair and
  is the hard bottleneck (the baseline's serialized DMA_INDIRECT stream).
  InstDMAGatherAnt (mlp ucode library, present on this image) packs up to
  ~1024 int16 indices per instruction, and instructions on different
  queue_num (0-3) run their desc-gen concurrently on different Q7 core
  pairs, giving ~4x the gather throughput.

  int16 indices limit each gather instruction to a 32768-row window of the
  table, so:
   - the user table (1M rows = 31 windows) is vocab-sharded: core c gets
     windows [4c, 4c+4) as its 131072-row shard, and processes the batch
     elements whose user index falls there (host-side all-to-all on
     indices; outputs unpermuted on the host afterwards).
   - the movie table (100K rows = 4 windows) is replicated; each core
     gathers its elements' movie rows with one instruction per
     (user-window x movie-window) cell so the rows land directly in the
     same canonical order as the user rows.
  Canonical per-core order: sort by (local user window, movie window); each
  cell padded to a multiple of 128 slots (dst slot k of a gather maps to
  partition k%128, column k//128 - cell alignment keeps both gathers'
  layouts identical).  One big DVE mul + reduce then computes all dots.
"""

import os
import numpy as np

N_USERS = 1_000_000
N_MOVIES = 100_000
EMB = 64
BATCH = 16384
N_CORES = 8
P = 128
W = 32768  # int16-addressable window rows
UW_PER_CORE = 4  # user windows per core (core 7 has 3 real + 1 dummy)
N_MW = 4  # movie windows (100000 / 32768 -> 4)
SHARD_ROWS = UW_PER_CORE * W  # 131072

_CACHE = {}


def _ceil(a, b):
    return -(-a // b)


def _plan(users, movies):
    """Host-side all-to-all + canonical ordering. Returns per-core plans and
    the uniform (cross-core max) instruction shapes."""
    uw = users // W  # 0..30
    core = np.minimum(uw // UW_PER_CORE, N_CORES - 1)
    mw = movies // W  # 0..3

    plans = []
    counts = np.zeros((N_CORES, UW_PER_CORE, N_MW), dtype=np.int64)
    for c in range(N_CORES):
        sel = np.flatnonzero(core == c)
        uwl = uw[sel] - c * UW_PER_CORE  # 0..3
        mwl = mw[sel]
        order = np.lexsort((mwl, uwl))
        elems = sel[order]
        uwl = uwl[order]
        mwl = mwl[order]
        for i in range(UW_PER_CORE):
            for j in range(N_MW):
                counts[c, i, j] = int(np.sum((uwl == i) & (mwl == j)))
        plans.append({"elems": elems, "uwl": uwl, "mwl": mwl})

    cnt_max = counts.max(axis=0)  # [4,4]
    cap = _ceil(cnt_max, 128) * 128  # slots per cell, uniform
    return plans, counts, cnt_max, cap


def _build_nc(cap, cnt_max):
    import concourse.bacc as bacc
    import concourse.tile as tile
    from concourse import mybir

    ncols = cap // 128  # [4,4] columns per cell
    C = int(ncols.sum())  # total columns
    # u-instruction list length per window i (sum of its cells' caps)
    Lu = [int(cap[i].sum()) for i in range(UW_PER_CORE)]
    assert all(l % 128 == 0 for l in Lu)
    for l in Lu:
        assert l <= 1024, f"u-instr num_idxs {l} > 1024 cap"
    # m-cell list lengths, padded to 16 for idx-tile slicing
    Lm16 = [[_ceil(int(cnt_max[i, j]), 16) * 16 for j in range(N_MW)] for i in range(UW_PER_CORE)]

    u_idx_cols = sum(Lu) // 16
    m_idx_cols = max(1, sum(sum(r) for r in Lm16) // 16)

    nc = bacc.Bacc(None, target_bir_lowering=False, num_swdge_queues=4)
    ushard_t = nc.dram_tensor("user_shard", [SHARD_ROWS, EMB], mybir.dt.float32, kind="ExternalInput")
    mtable_t = nc.dram_tensor("movie_table", [N_MOVIES, EMB], mybir.dt.float32, kind="ExternalInput")
    uidx_t = nc.dram_tensor("u_idx", [P, u_idx_cols], mybir.dt.int16, kind="ExternalInput")
    midx_t = nc.dram_tensor("m_idx", [P, m_idx_cols], mybir.dt.int16, kind="ExternalInput")
    out_t = nc.dram_tensor("out", [P, C], mybir.dt.float32, kind="ExternalOutput")

    with tile.TileContext(nc) as tc:
        with tc.tile_pool(name="sbuf", bufs=1) as sbuf:
            uidx = sbuf.tile([P, u_idx_cols], mybir.dt.int16)
            midx = sbuf.tile([P, m_idx_cols], mybir.dt.int16)
            nc.sync.dma_start(out=uidx[:], in_=uidx_t[:])
            nc.sync.dma_start(out=midx[:], in_=midx_t[:])

            U = sbuf.tile([P, C, EMB], mybir.dt.float32)
            M = sbuf.tile([P, C, EMB], mybir.dt.float32)

            # interleave gather issue across the 4 queues: u_i -> queue i,
            # m-cells round-robin so each queue's desc load stays balanced
            gathers = []  # (kind, i, j, queue, num_idxs, idx_off16, dst_col, n)
            uoff = 0
            for i in range(UW_PER_CORE):
                gathers.append(("u", i, None, i, Lu[i], uoff // 16, int(ncols[:i].sum() * N_MW and 0) ))
                uoff += Lu[i]
            # m-cells: queue = (i+j+...)%4 balanced by size later; simple rr
            moff = 0
            colbase = {}
            cb = 0
            for i in range(UW_PER_CORE):
                for j in range(N_MW):
                    colbase[(i, j)] = cb
                    cb += int(ncols[i, j])
            rr = 0
            for i in range(UW_PER_CORE):
                for j in range(N_MW):
                    n = int(cnt_max[i, j])
                    if n > 0:
                        gathers.append(("m", i, j, rr % 4, n, moff // 16, colbase[(i, j)]))
                        rr += 1
                    moff += Lm16[i][j]

            # issue u first (largest), then m-cells
            for g in gathers:
                kind, i, j, q, n, off16, _ = g
                if kind == "u":
                    dst_col = sum(int(ncols[i2].sum()) for i2 in range(i))
                    nl16 = Lu[i] // 16
                    nc.gpsimd.dma_gather(
                        out_ap=U[:, dst_col : dst_col + Lu[i] // 128],
                        in_ap=ushard_t[i * W : (i + 1) * W],
                        idxs_ap=uidx[:, off16 : off16 + nl16],
                        num_idxs=n,
                        num_idxs_reg=n,
                        elem_size=EMB,
                        queue_num=q,
                    )
                else:
                    dst_col = g[6]
                    ncol_ij = int(ncols[i, j])
                    nl16 = _ceil(n, 16)
                    ext = min(W, N_MOVIES - j * W)
                    nc.gpsimd.dma_gather(
                        out_ap=M[:, dst_col : dst_col + ncol_ij],
                        in_ap=mtable_t[j * W : j * W + ext],
                        idxs_ap=midx[:, off16 : off16 + nl16],
                        num_idxs=n,
                        num_idxs_reg=n,
                        elem_size=EMB,
                        queue_num=q,
                    )

            prod = sbuf.tile([P, C, EMB], mybir.dt.float32)
            nc.vector.tensor_mul(out=prod[:], in0=U[:], in1=M[:])
            res = sbuf.tile([P, C], mybir.dt.float32)
            nc.vector.tensor_reduce(
                out=res[:], in_=prod[:], axis=mybir.AxisListType.X, op=mybir.AluOpType.add
            )
            nc.sync.dma_start(out=out_t[:], in_=res[:])

    nc.compile()
    return nc, C, Lu, Lm16


def _wrap16(flat):
    """flat int16 list (len % 16 == 0) -> [128, len/16] wrapped + replicated."""
    n = flat.shape[0]
    blk = flat.reshape(n // 16, 16).T  # [16, n/16]
    return np.tile(blk, (8, 1))


def _install_ntff_hook():
    """Shim antenv.axon_hooks (absent in this image) so trace=True works
    under axon, and disable the S3 artifact upload (zero-egress container)."""
    import sys
    import types

    import concourse.bass_utils as bu

    bu.upload_artifacts = lambda d: d

    try:
        from antenv.axon_hooks import get_axon_ntff_profile_hook  # noqa: F401

        return
    except ImportError:
        pass

    import antenv
    from trn_agent_boot.trn_boot import _ntff_profile_via_ctypes

    mod = types.ModuleType("antenv.axon_hooks")
    mod._hook = _ntff_profile_via_ctypes("/opt/axon/libaxon_pjrt.so")
    mod.set_axon_ntff_profile_hook = lambda h: setattr(mod, "_hook", h)
    mod.get_axon_ntff_profile_hook = lambda: mod._hook
    sys.modules["antenv.axon_hooks"] = mod
    antenv.axon_hooks = mod


def kernel(users, movies, user_table, movie_table):
    from concourse.bass_utils import run_bass_kernel_spmd

    users = np.asarray(users).astype(np.int64)
    movies = np.asarray(movies).astype(np.int64)
    user_table = np.ascontiguousarray(np.asarray(user_table, dtype=np.float32))
    movie_table = np.ascontiguousarray(np.asarray(movie_table, dtype=np.float32))

    plans, counts, cnt_max, cap = _plan(users, movies)
    key = (tuple(cap.ravel()), tuple(cnt_max.ravel()))
    if _CACHE.get("key") != key:
        _CACHE["nc"], _CACHE["C"], _CACHE["Lu"], _CACHE["Lm16"] = _build_nc(cap, cnt_max)
        _CACHE["key"] = key
    nc, C, Lu, Lm16 = _CACHE["nc"], _CACHE["C"], _CACHE["Lu"], _CACHE["Lm16"]

    ncols = cap // 128
    colbase = {}
    cb = 0
    for i in range(UW_PER_CORE):
        for j in range(N_MW):
            colbase[(i, j)] = cb
            cb += int(ncols[i, j])

    in_maps = []
    slot_maps = []  # per core: (batch_ids, slots)
    for c in range(N_CORES):
        pl = plans[c]
        elems, uwl, mwl = pl["elems"], pl["uwl"], pl["mwl"]

        # user shard (core 7's 4th window is zeros padding)
        base = c * SHARD_ROWS
        end = min(N_USERS, base + SHARD_ROWS)
        shard = np.zeros((SHARD_ROWS, EMB), dtype=np.float32)
        shard[: end - base] = user_table[base:end]

        u_flat = []
        m_flat = []
        batch_ids = []
        slots = []
        for i in range(UW_PER_CORE):
            for j in range(N_MW):
                cell = elems[(uwl == i) & (mwl == j)]
                cnt = cell.shape[0]
                capij = int(cap[i, j])
                # u list: local idx + dummy-0 pads to cap
                ul = np.zeros(capij, dtype=np.int16)
                ul[:cnt] = (users[cell] - (c * UW_PER_CORE + i) * W).astype(np.int16)
                u_flat.append(ul)
                # m list: local idx, -1 pads to uniform cnt_max then to 16
                l16 = Lm16[i][j]
                # all pads are valid dummy index 0: truncating lists (trailing
                # -1) to few/zero descriptors hard-crashes the gather ucode
                ml = np.zeros(l16, dtype=np.int16)
                ml[:cnt] = (movies[cell] - j * W).astype(np.int16)
                m_flat.append(ml)
                # slots for unpermute
                s0 = colbase[(i, j)] * 128
                slots.append(s0 + np.arange(cnt))
                batch_ids.append(cell)
        u_flat = np.concatenate(u_flat)
        m_flat = np.concatenate(m_flat) if m_flat else np.zeros(16, np.int16)
        slot_maps.append((np.concatenate(batch_ids), np.concatenate(slots)))

        in_maps.append(
            {
                "user_shard": shard,
                "movie_table": movie_table,
                "u_idx": np.ascontiguousarray(_wrap16(u_flat)),
                "m_idx": np.ascontiguousarray(_wrap16(m_flat)),
            }
        )

    trace = bool(os.environ.get("KERNEL_TRACE"))
    if trace:
        try:
            _install_ntff_hook()
        except Exception:
            trace = False
    res = run_bass_kernel_spmd(nc, in_maps, core_ids=list(range(N_CORES)), trace=trace)
    if trace:
        kernel.last_exec_time_ns = res.exec_time_ns
        kernel.last_trace = res.instructions_and_trace

    out = np.zeros((BATCH,), dtype=np.float32)
    for c in range(N_CORES):
        r = res.results[c]["out"]  # [128, C]
        batch_ids, slots = slot_maps[c]
        # slot s -> partition s%128, col s//128
        out[batch_ids] = r[slots % 128, slots // 128]
    return out.reshape(BATCH, 1)


# revision 5
# speedup vs baseline: 1.5709x; 1.5709x over previous
"""Embedding-lookup + row-wise dot kernel for Trainium2 (8 NeuronCores).

Problem (hardcoded, self-contained):
    users:       [16384] int   (values < 1_000_000)
    movies:      [16384] int   (values < 100_000)
    user_table:  [1_000_000, 64] f32
    movie_table: [100_000, 64] f32
    out = sum(user_table[users] * movie_table[movies], axis=-1, keepdims=True)

Strategy (v3 — 4-queue InstDMAGatherAnt, raw Block mode):
  SWDGE Q7 desc-gen (~10ns/desc per queue pair) is the hard bottleneck; the
  ant dma_gather packs ~hundreds of int16 indices per instruction and
  different queue_num (0-3) desc-gen concurrently on different Q7 pairs
  (~4x).  int16 limits one instruction to a 32768-row table window, so the
  user table is vocab-sharded (core c owns windows [4c,4c+4)) with a
  host-side all-to-all on indices, and the movie table is replicated with
  one gather per (user-window x movie-window) cell so rows land directly in
  canonical order.  Raw Block mode with a single cumulative DMA semaphore
  avoids the Tile framework's small DMA-sem pool, whose reuse waits
  serialize a 32-gather stream.
"""

import os
import numpy as np

N_USERS = 1_000_000
N_MOVIES = 100_000
EMB = 64
BATCH = 16384
N_CORES = 8
P = 128
W = 32768
UW_PER_CORE = 4
N_MW = 4
SHARD_ROWS = UW_PER_CORE * W  # 131072

_CACHE = {}


def _ceil(a, b):
    return -(-a // b)


def _plan(users, movies):
    uw = users // W
    core = np.minimum(uw // UW_PER_CORE, N_CORES - 1)
    mw = movies // W

    plans = []
    counts = np.zeros((N_CORES, UW_PER_CORE, N_MW), dtype=np.int64)
    for c in range(N_CORES):
        sel = np.flatnonzero(core == c)
        uwl = uw[sel] - c * UW_PER_CORE
        mwl = mw[sel]
        order = np.lexsort((mwl, uwl))
        elems = sel[order]
        uwl = uwl[order]
        mwl = mwl[order]
        for i in range(UW_PER_CORE):
            for j in range(N_MW):
                counts[c, i, j] = int(np.sum((uwl == i) & (mwl == j)))
        plans.append({"elems": elems, "uwl": uwl, "mwl": mwl})

    cnt_max = counts.max(axis=0)  # [4,4] uniform per-cell instruction size
    cap = _ceil(np.maximum(cnt_max, 1), 128) * 128  # slots per cell
    return plans, counts, cnt_max, cap


def _build_nc(cap, cnt_max):
    import concourse.bacc as bacc
    from concourse import mybir
    from concourse.library_config import mlp

    ncols = cap // 128
    C = int(ncols.sum())
    # idx list layout: per cell, u and m lists both padded to 16 (and the
    # dst capacity to 128); every pad is a valid dummy index 0 (all-(-1)
    # truncation crashes the ucode, and cnt<16 lists are untested territory)
    L16 = [[max(16, _ceil(int(cnt_max[i, j]), 16) * 16) for j in range(N_MW)] for i in range(UW_PER_CORE)]
    idx_cols = sum(sum(r) for r in L16) // 16

    nc = bacc.Bacc(None, target_bir_lowering=False, num_swdge_queues=4)
    ushard_t = nc.dram_tensor("user_shard", [SHARD_ROWS, EMB], mybir.dt.float32, kind="ExternalInput")
    mtable_t = nc.dram_tensor("movie_table", [N_MOVIES, EMB], mybir.dt.float32, kind="ExternalInput")
    uidx_t = nc.dram_tensor("u_idx", [P, idx_cols], mybir.dt.int16, kind="ExternalInput")
    midx_t = nc.dram_tensor("m_idx", [P, idx_cols], mybir.dt.int16, kind="ExternalInput")
    out_t = nc.dram_tensor("out", [P, C], mybir.dt.float32, kind="ExternalOutput")

    n_gather = 2 * UW_PER_CORE * N_MW

    with (
        nc.Block() as block,
        nc.sbuf_tensor("uidx", [P, idx_cols], mybir.dt.int16) as uidx,
        nc.sbuf_tensor("midx", [P, idx_cols], mybir.dt.int16) as midx,
        nc.sbuf_tensor("U", [P, C, EMB], mybir.dt.float32) as U,
        nc.sbuf_tensor("M", [P, C, EMB], mybir.dt.float32) as M,
        nc.sbuf_tensor("prod", [P, C, EMB], mybir.dt.float32) as prod,
        nc.sbuf_tensor("res", [P, C], mybir.dt.float32) as res,
        nc.semaphore("idx_sem") as idx_sem,
        nc.semaphore("gat_sem") as gat_sem,
        nc.semaphore("cmp_sem") as cmp_sem,
        nc.semaphore("out_sem") as out_sem,
    ):
        # gather schedule: (kind, i, j, queue) round-robin across queues,
        # cells interleaved so each queue's desc load is balanced
        sched = []
        rr = 0
        for i in range(UW_PER_CORE):
            for j in range(N_MW):
                sched.append(("u", i, j, rr % 4))
                rr += 1
                sched.append(("m", i, j, rr % 4))
                rr += 1

        colbase = {}
        cb = 0
        off16 = {}
        o = 0
        for i in range(UW_PER_CORE):
            for j in range(N_MW):
                colbase[(i, j)] = cb
                cb += int(ncols[i, j])
                off16[(i, j)] = o
                o += L16[i][j] // 16

        @block.sync
        def _(sync):
            sync.dma_start(out=uidx[:], in_=uidx_t[:]).then_inc(idx_sem, 16)
            sync.dma_start(out=midx[:], in_=midx_t[:]).then_inc(idx_sem, 16)
            sync.wait_ge(cmp_sem, 1)
            sync.dma_start(out=out_t[:], in_=res[:]).then_inc(out_sem, 16)
            sync.wait_ge(out_sem, 16)

        @block.gpsimd
        def _(gpsimd):
            gpsimd.load_library(mlp)
            gpsimd.wait_ge(idx_sem, 32)
            for kind, i, j, q in sched:
                n = max(16, int(cnt_max[i, j]))
                dst_col = colbase[(i, j)]
                ncol_ij = int(ncols[i, j])
                o16 = off16[(i, j)]
                nl16 = L16[i][j] // 16
                if kind == "u":
                    gpsimd.dma_gather(
                        out_ap=U[:, dst_col : dst_col + ncol_ij],
                        in_ap=ushard_t[i * W : (i + 1) * W],
                        idxs_ap=uidx[:, o16 : o16 + nl16],
                        num_idxs=n,
                        num_idxs_reg=n,
                        elem_size=EMB,
                        queue_num=q,
                    ).then_inc(gat_sem, 16)
                else:
                    ext = min(W, N_MOVIES - j * W)
                    gpsimd.dma_gather(
                        out_ap=M[:, dst_col : dst_col + ncol_ij],
                        in_ap=mtable_t[j * W : j * W + ext],
                        idxs_ap=midx[:, o16 : o16 + nl16],
                        num_idxs=n,
                        num_idxs_reg=n,
                        elem_size=EMB,
                        queue_num=q,
                    ).then_inc(gat_sem, 16)

        @block.vector
        def _(vector):
            vector.wait_ge(gat_sem, 16 * n_gather)
            vector.tensor_mul(out=prod[:], in0=U[:], in1=M[:])
            vector.tensor_reduce(
                out=res[:], in_=prod[:], axis=mybir.AxisListType.X, op=mybir.AluOpType.add
            ).then_inc(cmp_sem, 1)

    nc.compile()
    return nc, C, L16


def _wrap16(flat):
    n = flat.shape[0]
    blk = flat.reshape(n // 16, 16).T
    return np.tile(blk, (8, 1))


def _install_ntff_hook():
    """Shim antenv.axon_hooks (absent in this image) so trace=True works
    under axon, and disable the S3 artifact upload (zero-egress container)."""
    import sys
    import types

    import concourse.bass_utils as bu

    bu.upload_artifacts = lambda d: d

    try:
        from antenv.axon_hooks import get_axon_ntff_profile_hook  # noqa: F401

        return
    except ImportError:
        pass

    import antenv
    from trn_agent_boot.trn_boot import _ntff_profile_via_ctypes

    mod = types.ModuleType("antenv.axon_hooks")
    mod._hook = _ntff_profile_via_ctypes("/opt/axon/libaxon_pjrt.so")
    mod.set_axon_ntff_profile_hook = lambda h: setattr(mod, "_hook", h)
    mod.get_axon_ntff_profile_hook = lambda: mod._hook
    sys.modules["antenv.axon_hooks"] = mod
    antenv.axon_hooks = mod


def kernel(users, movies, user_table, movie_table):
    from concourse.bass_utils import run_bass_kernel_spmd

    users = np.asarray(users).astype(np.int64)
    movies = np.asarray(movies).astype(np.int64)
    user_table = np.ascontiguousarray(np.asarray(user_table, dtype=np.float32))
    movie_table = np.ascontiguousarray(np.asarray(movie_table, dtype=np.float32))

    plans, counts, cnt_max, cap = _plan(users, movies)
    key = (tuple(cap.ravel()), tuple(cnt_max.ravel()))
    if _CACHE.get("key") != key:
        _CACHE["nc"], _CACHE["C"], _CACHE["L16"] = _build_nc(cap, cnt_max)
        _CACHE["key"] = key
    nc, C, L16 = _CACHE["nc"], _CACHE["C"], _CACHE["L16"]

    ncols = cap // 128
    colbase = {}
    cb = 0
    for i in range(UW_PER_CORE):
        for j in range(N_MW):
            colbase[(i, j)] = cb
            cb += int(ncols[i, j])

    in_maps = []
    slot_maps = []
    for c in range(N_CORES):
        pl = plans[c]
        elems, uwl, mwl = pl["elems"], pl["uwl"], pl["mwl"]

        base = c * SHARD_ROWS
        end = min(N_USERS, base + SHARD_ROWS)
        shard = np.zeros((SHARD_ROWS, EMB), dtype=np.float32)
        shard[: end - base] = user_table[base:end]

        u_flat = []
        m_flat = []
        batch_ids = []
        slots = []
        for i in range(UW_PER_CORE):
            for j in range(N_MW):
                cell = elems[(uwl == i) & (mwl == j)]
                cnt = cell.shape[0]
                l16 = L16[i][j]
                ul = np.zeros(l16, dtype=np.int16)
                ul[:cnt] = (users[cell] - (c * UW_PER_CORE + i) * W).astype(np.int16)
                u_flat.append(ul)
                ml = np.zeros(l16, dtype=np.int16)
                ml[:cnt] = (movies[cell] - j * W).astype(np.int16)
                m_flat.append(ml)
                s0 = colbase[(i, j)] * 128
                slots.append(s0 + np.arange(cnt))
                batch_ids.append(cell)
        u_flat = np.concatenate(u_flat)
        m_flat = np.concatenate(m_flat)
        slot_maps.append((np.concatenate(batch_ids), np.concatenate(slots)))

        in_maps.append(
            {
                "user_shard": shard,
                "movie_table": movie_table,
                "u_idx": np.ascontiguousarray(_wrap16(u_flat)),
                "m_idx": np.ascontiguousarray(_wrap16(m_flat)),
            }
        )

    trace = os.environ.get("KERNEL_TRACE", "") not in ("", "0")
    if trace:
        try:
            _install_ntff_hook()
        except Exception:
            trace = False
    res = run_bass_kernel_spmd(nc, in_maps, core_ids=list(range(N_CORES)), trace=trace)
    if trace:
        kernel.last_exec_time_ns = res.exec_time_ns
        kernel.last_trace = res.instructions_and_trace

    out = np.zeros((BATCH,), dtype=np.float32)
    for c in range(N_CORES):
        r = res.results[c]["out"]
        batch_ids, slots = slot_maps[c]
        out[batch_ids] = r[slots % 128, slots // 128]
    return out.reshape(BATCH, 1)


# revision 8
# speedup vs baseline: 1.6565x; 1.0545x over previous
"""Embedding-lookup + row-wise dot kernel for Trainium2 (8 NeuronCores).

Problem (hardcoded, self-contained):
    users:       [16384] int   (values < 1_000_000)
    movies:      [16384] int   (values < 100_000)
    user_table:  [1_000_000, 64] f32
    movie_table: [100_000, 64] f32
    out = sum(user_table[users] * movie_table[movies], axis=-1, keepdims=True)

Strategy (v3 — 4-queue InstDMAGatherAnt, raw Block mode):
  SWDGE Q7 desc-gen (~10ns/desc per queue pair) is the hard bottleneck; the
  ant dma_gather packs ~hundreds of int16 indices per instruction and
  different queue_num (0-3) desc-gen concurrently on different Q7 pairs
  (~4x).  int16 limits one instruction to a 32768-row table window, so the
  user table is vocab-sharded (core c owns windows [4c,4c+4)) with a
  host-side all-to-all on indices, and the movie table is replicated with
  one gather per (user-window x movie-window) cell so rows land directly in
  canonical order.  Raw Block mode with a single cumulative DMA semaphore
  avoids the Tile framework's small DMA-sem pool, whose reuse waits
  serialize a 32-gather stream.
"""

import os
import numpy as np

N_USERS = 1_000_000
N_MOVIES = 100_000
EMB = 64
BATCH = 16384
N_CORES = 8
P = 128
W = 32768
UW_PER_CORE = 4
N_MW = 4
SHARD_ROWS = UW_PER_CORE * W  # 131072

_CACHE = {}


def _ceil(a, b):
    return -(-a // b)


def _plan(users, movies):
    uw = users // W
    core = np.minimum(uw // UW_PER_CORE, N_CORES - 1)
    mw = movies // W

    plans = []
    counts = np.zeros((N_CORES, UW_PER_CORE, N_MW), dtype=np.int64)
    for c in range(N_CORES):
        sel = np.flatnonzero(core == c)
        uwl = uw[sel] - c * UW_PER_CORE
        mwl = mw[sel]
        order = np.lexsort((mwl, uwl))
        elems = sel[order]
        uwl = uwl[order]
        mwl = mwl[order]
        for i in range(UW_PER_CORE):
            for j in range(N_MW):
                counts[c, i, j] = int(np.sum((uwl == i) & (mwl == j)))
        plans.append({"elems": elems, "uwl": uwl, "mwl": mwl})

    cnt_max = counts.max(axis=0)  # [4,4] uniform per-cell instruction size
    cap = _ceil(np.maximum(cnt_max, 1), 128) * 128  # slots per cell
    return plans, counts, cnt_max, cap


def _build_nc(cap, cnt_max):
    import concourse.bacc as bacc
    from concourse import mybir
    from concourse.library_config import mlp

    ncols = cap // 128
    C = int(ncols.sum())
    # idx list layout: per cell, u and m lists both padded to 16 (and the
    # dst capacity to 128); every pad is a valid dummy index 0 (all-(-1)
    # truncation crashes the ucode, and cnt<16 lists are untested territory)
    L16 = [[max(16, _ceil(int(cnt_max[i, j]), 16) * 16) for j in range(N_MW)] for i in range(UW_PER_CORE)]
    idx_cols = sum(sum(r) for r in L16) // 16

    nc = bacc.Bacc(None, target_bir_lowering=False, num_swdge_queues=4)
    ushard_t = nc.dram_tensor("user_shard", [SHARD_ROWS, EMB], mybir.dt.float32, kind="ExternalInput")
    mtable_t = nc.dram_tensor("movie_table", [N_MOVIES, EMB], mybir.dt.float32, kind="ExternalInput")
    uidx_t = nc.dram_tensor("u_idx", [P, idx_cols], mybir.dt.int16, kind="ExternalInput")
    midx_t = nc.dram_tensor("m_idx", [P, idx_cols], mybir.dt.int16, kind="ExternalInput")
    out_t = nc.dram_tensor("out", [P, C], mybir.dt.float32, kind="ExternalOutput")

    n_gather = 2 * UW_PER_CORE * N_MW

    with (
        nc.Block() as block,
        nc.sbuf_tensor("uidx", [P, idx_cols], mybir.dt.int16) as uidx,
        nc.sbuf_tensor("midx", [P, idx_cols], mybir.dt.int16) as midx,
        nc.sbuf_tensor("U", [P, C, EMB], mybir.dt.float32) as U,
        nc.sbuf_tensor("M", [P, C, EMB], mybir.dt.float32) as M,
        nc.sbuf_tensor("prod", [P, C, EMB], mybir.dt.float32) as prod,
        nc.sbuf_tensor("res", [P, C], mybir.dt.float32) as res,
        nc.semaphore("idx_sem") as idx_sem,
        nc.semaphore("gat0") as gat0,
        nc.semaphore("gat1") as gat1,
        nc.semaphore("gat2") as gat2,
        nc.semaphore("gat3") as gat3,
        nc.semaphore("cmp_sem") as cmp_sem,
        nc.semaphore("out_sem") as out_sem,
    ):
        gat_sems = [gat0, gat1, gat2, gat3]
        # gather schedule: (kind, i, j, queue) round-robin across queues,
        # cells interleaved so each queue's desc load is balanced
        sched = []
        rr = 0
        for i in range(UW_PER_CORE):
            for j in range(N_MW):
                sched.append(("u", i, j, rr % 4))
                rr += 1
                sched.append(("m", i, j, rr % 4))
                rr += 1

        colbase = {}
        cb = 0
        off16 = {}
        o = 0
        for i in range(UW_PER_CORE):
            for j in range(N_MW):
                colbase[(i, j)] = cb
                cb += int(ncols[i, j])
                off16[(i, j)] = o
                o += L16[i][j] // 16

        @block.sync
        def _(sync):
            sync.dma_start(out=uidx[:], in_=uidx_t[:]).then_inc(idx_sem, 16)
            sync.dma_start(out=midx[:], in_=midx_t[:]).then_inc(idx_sem, 16)
            sync.wait_ge(cmp_sem, UW_PER_CORE)
            sync.dma_start(out=out_t[:], in_=res[:]).then_inc(out_sem, 16)
            sync.wait_ge(out_sem, 16)

        @block.gpsimd
        def _(gpsimd):
            gpsimd.load_library(mlp)
            gpsimd.wait_ge(idx_sem, 32)
            for kind, i, j, q in sched:
                n = max(16, int(cnt_max[i, j]))
                dst_col = colbase[(i, j)]
                ncol_ij = int(ncols[i, j])
                o16 = off16[(i, j)]
                nl16 = L16[i][j] // 16
                if kind == "u":
                    gpsimd.dma_gather(
                        out_ap=U[:, dst_col : dst_col + ncol_ij],
                        in_ap=ushard_t[i * W : (i + 1) * W],
                        idxs_ap=uidx[:, o16 : o16 + nl16],
                        num_idxs=n,
                        num_idxs_reg=n,
                        elem_size=EMB,
                        queue_num=q,
                    ).then_inc(gat_sems[i], 16)
                else:
                    ext = min(W, N_MOVIES - j * W)
                    gpsimd.dma_gather(
                        out_ap=M[:, dst_col : dst_col + ncol_ij],
                        in_ap=mtable_t[j * W : j * W + ext],
                        idxs_ap=midx[:, o16 : o16 + nl16],
                        num_idxs=n,
                        num_idxs_reg=n,
                        elem_size=EMB,
                        queue_num=q,
                    ).then_inc(gat_sems[i], 16)

        @block.vector
        def _(vector):
            # pipelined: window i's mul+reduce runs as soon as its 8 gathers land
            for i in range(UW_PER_CORE):
                c0 = colbase[(i, 0)]
                nc_i = sum(int(ncols[i, j]) for j in range(N_MW))
                vector.wait_ge(gat_sems[i], 16 * 2 * N_MW)
                vector.tensor_mul(
                    out=prod[:, c0 : c0 + nc_i], in0=U[:, c0 : c0 + nc_i], in1=M[:, c0 : c0 + nc_i]
                )
                vector.tensor_reduce(
                    out=res[:, c0 : c0 + nc_i],
                    in_=prod[:, c0 : c0 + nc_i],
                    axis=mybir.AxisListType.X,
                    op=mybir.AluOpType.add,
                ).then_inc(cmp_sem, 1)

    nc.compile()
    return nc, C, L16


def _wrap16(flat):
    n = flat.shape[0]
    blk = flat.reshape(n // 16, 16).T
    return np.tile(blk, (8, 1))


def _install_ntff_hook():
    """Shim antenv.axon_hooks (absent in this image) so trace=True works
    under axon, and disable the S3 artifact upload (zero-egress container)."""
    import sys
    import types

    import concourse.bass_utils as bu

    bu.upload_artifacts = lambda d: d

    try:
        from antenv.axon_hooks import get_axon_ntff_profile_hook  # noqa: F401

        return
    except ImportError:
        pass

    import antenv
    from trn_agent_boot.trn_boot import _ntff_profile_via_ctypes

    mod = types.ModuleType("antenv.axon_hooks")
    mod._hook = _ntff_profile_via_ctypes("/opt/axon/libaxon_pjrt.so")
    mod.set_axon_ntff_profile_hook = lambda h: setattr(mod, "_hook", h)
    mod.get_axon_ntff_profile_hook = lambda: mod._hook
    sys.modules["antenv.axon_hooks"] = mod
    antenv.axon_hooks = mod


def kernel(users, movies, user_table, movie_table):
    from concourse.bass_utils import run_bass_kernel_spmd

    users = np.asarray(users).astype(np.int64)
    movies = np.asarray(movies).astype(np.int64)
    user_table = np.ascontiguousarray(np.asarray(user_table, dtype=np.float32))
    movie_table = np.ascontiguousarray(np.asarray(movie_table, dtype=np.float32))

    plans, counts, cnt_max, cap = _plan(users, movies)
    key = (tuple(cap.ravel()), tuple(cnt_max.ravel()))
    if _CACHE.get("key") != key:
        _CACHE["nc"], _CACHE["C"], _CACHE["L16"] = _build_nc(cap, cnt_max)
        _CACHE["key"] = key
    nc, C, L16 = _CACHE["nc"], _CACHE["C"], _CACHE["L16"]

    ncols = cap // 128
    colbase = {}
    cb = 0
    for i in range(UW_PER_CORE):
        for j in range(N_MW):
            colbase[(i, j)] = cb
            cb += int(ncols[i, j])

    in_maps = []
    slot_maps = []
    for c in range(N_CORES):
        pl = plans[c]
        elems, uwl, mwl = pl["elems"], pl["uwl"], pl["mwl"]

        base = c * SHARD_ROWS
        end = min(N_USERS, base + SHARD_ROWS)
        shard = np.zeros((SHARD_ROWS, EMB), dtype=np.float32)
        shard[: end - base] = user_table[base:end]

        u_flat = []
        m_flat = []
        batch_ids = []
        slots = []
        for i in range(UW_PER_CORE):
            for j in range(N_MW):
                cell = elems[(uwl == i) & (mwl == j)]
                cnt = cell.shape[0]
                l16 = L16[i][j]
                ul = np.zeros(l16, dtype=np.int16)
                ul[:cnt] = (users[cell] - (c * UW_PER_CORE + i) * W).astype(np.int16)
                u_flat.append(ul)
                ml = np.zeros(l16, dtype=np.int16)
                ml[:cnt] = (movies[cell] - j * W).astype(np.int16)
                m_flat.append(ml)
                s0 = colbase[(i, j)] * 128
                slots.append(s0 + np.arange(cnt))
                batch_ids.append(cell)
        u_flat = np.concatenate(u_flat)
        m_flat = np.concatenate(m_flat)
        slot_maps.append((np.concatenate(batch_ids), np.concatenate(slots)))

        in_maps.append(
            {
                "user_shard": shard,
                "movie_table": movie_table,
                "u_idx": np.ascontiguousarray(_wrap16(u_flat)),
                "m_idx": np.ascontiguousarray(_wrap16(m_flat)),
            }
        )

    trace = os.environ.get("KERNEL_TRACE", "") not in ("", "0")
    if trace:
        try:
            _install_ntff_hook()
        except Exception:
            trace = False
    res = run_bass_kernel_spmd(nc, in_maps, core_ids=list(range(N_CORES)), trace=trace)
    if trace:
        kernel.last_exec_time_ns = res.exec_time_ns
        kernel.last_trace = res.instructions_and_trace

    out = np.zeros((BATCH,), dtype=np.float32)
    for c in range(N_CORES):
        r = res.results[c]["out"]
        batch_ids, slots = slot_maps[c]
        out[batch_ids] = r[slots % 128, slots // 128]
    return out.reshape(BATCH, 1)


# revision 9
# speedup vs baseline: 1.6595x; 1.0018x over previous
"""Embedding-lookup + row-wise dot kernel for Trainium2 (8 NeuronCores).

Problem (hardcoded, self-contained):
    users:       [16384] int   (values < 1_000_000)
    movies:      [16384] int   (values < 100_000)
    user_table:  [1_000_000, 64] f32
    movie_table: [100_000, 64] f32
    out = sum(user_table[users] * movie_table[movies], axis=-1, keepdims=True)

Strategy (v3 — 4-queue InstDMAGatherAnt, raw Block mode):
  SWDGE Q7 desc-gen (~10ns/desc per queue pair) is the hard bottleneck; the
  ant dma_gather packs ~hundreds of int16 indices per instruction and
  different queue_num (0-3) desc-gen concurrently on different Q7 pairs
  (~4x).  int16 limits one instruction to a 32768-row table window, so the
  user table is vocab-sharded (core c owns windows [4c,4c+4)) with a
  host-side all-to-all on indices, and the movie table is replicated with
  one gather per (user-window x movie-window) cell so rows land directly in
  canonical order.  Raw Block mode with a single cumulative DMA semaphore
  avoids the Tile framework's small DMA-sem pool, whose reuse waits
  serialize a 32-gather stream.
"""

import os
import numpy as np

N_USERS = 1_000_000
N_MOVIES = 100_000
EMB = 64
BATCH = 16384
N_CORES = 8
P = 128
W = 32768
UW_PER_CORE = 4
N_MW = 4
SHARD_ROWS = UW_PER_CORE * W  # 131072

_CACHE = {}


def _ceil(a, b):
    return -(-a // b)


def _plan(users, movies):
    uw = users // W
    core = np.minimum(uw // UW_PER_CORE, N_CORES - 1)
    mw = movies // W

    plans = []
    counts = np.zeros((N_CORES, UW_PER_CORE, N_MW), dtype=np.int64)
    for c in range(N_CORES):
        sel = np.flatnonzero(core == c)
        uwl = uw[sel] - c * UW_PER_CORE
        mwl = mw[sel]
        order = np.lexsort((mwl, uwl))
        elems = sel[order]
        uwl = uwl[order]
        mwl = mwl[order]
        for i in range(UW_PER_CORE):
            for j in range(N_MW):
                counts[c, i, j] = int(np.sum((uwl == i) & (mwl == j)))
        plans.append({"elems": elems, "uwl": uwl, "mwl": mwl})

    cnt_max = counts.max(axis=0)  # [4,4] uniform per-cell instruction size
    # ucode caps one dma_gather at ~1024 indices (4KB Q7 idx scratch)
    assert int(cnt_max.max()) <= 1024, f"cell count {int(cnt_max.max())} > 1024"
    cap = _ceil(np.maximum(cnt_max, 1), 128) * 128  # slots per cell
    return plans, counts, cnt_max, cap


def _build_nc(cap, cnt_max):
    import concourse.bacc as bacc
    from concourse import mybir
    from concourse.library_config import mlp

    ncols = cap // 128
    C = int(ncols.sum())
    # idx list layout: per cell, u and m lists both padded to 16 (and the
    # dst capacity to 128); every pad is a valid dummy index 0 (all-(-1)
    # truncation crashes the ucode, and cnt<16 lists are untested territory)
    L16 = [[max(16, _ceil(int(cnt_max[i, j]), 16) * 16) for j in range(N_MW)] for i in range(UW_PER_CORE)]
    idx_cols = sum(sum(r) for r in L16) // 16

    nc = bacc.Bacc(None, target_bir_lowering=False, num_swdge_queues=4)
    ushard_t = nc.dram_tensor("user_shard", [SHARD_ROWS, EMB], mybir.dt.float32, kind="ExternalInput")
    mtable_t = nc.dram_tensor("movie_table", [N_MOVIES, EMB], mybir.dt.float32, kind="ExternalInput")
    uidx_t = nc.dram_tensor("u_idx", [P, idx_cols], mybir.dt.int16, kind="ExternalInput")
    midx_t = nc.dram_tensor("m_idx", [P, idx_cols], mybir.dt.int16, kind="ExternalInput")
    out_t = nc.dram_tensor("out", [P, C], mybir.dt.float32, kind="ExternalOutput")

    n_gather = 2 * UW_PER_CORE * N_MW

    with (
        nc.Block() as block,
        nc.sbuf_tensor("uidx", [P, idx_cols], mybir.dt.int16) as uidx,
        nc.sbuf_tensor("midx", [P, idx_cols], mybir.dt.int16) as midx,
        nc.sbuf_tensor("U", [P, C, EMB], mybir.dt.float32) as U,
        nc.sbuf_tensor("M", [P, C, EMB], mybir.dt.float32) as M,
        nc.sbuf_tensor("prod", [P, C, EMB], mybir.dt.float32) as prod,
        nc.sbuf_tensor("res", [P, C], mybir.dt.float32) as res,
        nc.semaphore("idx_sem") as idx_sem,
        nc.semaphore("gat0") as gat0,
        nc.semaphore("gat1") as gat1,
        nc.semaphore("gat2") as gat2,
        nc.semaphore("gat3") as gat3,
        nc.semaphore("cmp_sem") as cmp_sem,
        nc.semaphore("out_sem") as out_sem,
    ):
        gat_sems = [gat0, gat1, gat2, gat3]
        # gather schedule: (kind, i, j, queue) round-robin across queues,
        # cells interleaved so each queue's desc load is balanced
        sched = []
        rr = 0
        for i in range(UW_PER_CORE):
            for j in range(N_MW):
                sched.append(("u", i, j, rr % 4))
                rr += 1
                sched.append(("m", i, j, rr % 4))
                rr += 1

        colbase = {}
        cb = 0
        off16 = {}
        o = 0
        for i in range(UW_PER_CORE):
            for j in range(N_MW):
                colbase[(i, j)] = cb
                cb += int(ncols[i, j])
                off16[(i, j)] = o
                o += L16[i][j] // 16

        @block.sync
        def _(sync):
            sync.dma_start(out=uidx[:], in_=uidx_t[:]).then_inc(idx_sem, 16)
            sync.dma_start(out=midx[:], in_=midx_t[:]).then_inc(idx_sem, 16)
            sync.wait_ge(cmp_sem, UW_PER_CORE)
            sync.dma_start(out=out_t[:], in_=res[:]).then_inc(out_sem, 16)
            sync.wait_ge(out_sem, 16)

        @block.gpsimd
        def _(gpsimd):
            gpsimd.load_library(mlp)
            gpsimd.wait_ge(idx_sem, 32)
            for kind, i, j, q in sched:
                n = max(16, int(cnt_max[i, j]))
                dst_col = colbase[(i, j)]
                ncol_ij = int(ncols[i, j])
                o16 = off16[(i, j)]
                nl16 = L16[i][j] // 16
                if kind == "u":
                    gpsimd.dma_gather(
                        out_ap=U[:, dst_col : dst_col + ncol_ij],
                        in_ap=ushard_t[i * W : (i + 1) * W],
                        idxs_ap=uidx[:, o16 : o16 + nl16],
                        num_idxs=n,
                        num_idxs_reg=n,
                        elem_size=EMB,
                        queue_num=q,
                    ).then_inc(gat_sems[i], 16)
                else:
                    ext = min(W, N_MOVIES - j * W)
                    gpsimd.dma_gather(
                        out_ap=M[:, dst_col : dst_col + ncol_ij],
                        in_ap=mtable_t[j * W : j * W + ext],
                        idxs_ap=midx[:, o16 : o16 + nl16],
                        num_idxs=n,
                        num_idxs_reg=n,
                        elem_size=EMB,
                        queue_num=q,
                    ).then_inc(gat_sems[i], 16)

        @block.vector
        def _(vector):
            # pipelined: window i's mul+reduce runs as soon as its 8 gathers land
            for i in range(UW_PER_CORE):
                c0 = colbase[(i, 0)]
                nc_i = sum(int(ncols[i, j]) for j in range(N_MW))
                vector.wait_ge(gat_sems[i], 16 * 2 * N_MW)
                vector.tensor_mul(
                    out=prod[:, c0 : c0 + nc_i], in0=U[:, c0 : c0 + nc_i], in1=M[:, c0 : c0 + nc_i]
                )
                vector.tensor_reduce(
                    out=res[:, c0 : c0 + nc_i],
                    in_=prod[:, c0 : c0 + nc_i],
                    axis=mybir.AxisListType.X,
                    op=mybir.AluOpType.add,
                ).then_inc(cmp_sem, 1)

    nc.compile()
    return nc, C, L16


def _wrap16(flat):
    n = flat.shape[0]
    blk = flat.reshape(n // 16, 16).T
    return np.tile(blk, (8, 1))


def _install_ntff_hook():
    """Shim antenv.axon_hooks (absent in this image) so trace=True works
    under axon, and disable the S3 artifact upload (zero-egress container)."""
    import sys
    import types

    import concourse.bass_utils as bu

    bu.upload_artifacts = lambda d: d

    try:
        from antenv.axon_hooks import get_axon_ntff_profile_hook  # noqa: F401

        return
    except ImportError:
        pass

    import antenv
    from trn_agent_boot.trn_boot import _ntff_profile_via_ctypes

    mod = types.ModuleType("antenv.axon_hooks")
    mod._hook = _ntff_profile_via_ctypes("/opt/axon/libaxon_pjrt.so")
    mod.set_axon_ntff_profile_hook = lambda h: setattr(mod, "_hook", h)
    mod.get_axon_ntff_profile_hook = lambda: mod._hook
    sys.modules["antenv.axon_hooks"] = mod
    antenv.axon_hooks = mod


def kernel(users, movies, user_table, movie_table):
    from concourse.bass_utils import run_bass_kernel_spmd

    users = np.asarray(users).astype(np.int64)
    movies = np.asarray(movies).astype(np.int64)
    user_table = np.ascontiguousarray(np.asarray(user_table, dtype=np.float32))
    movie_table = np.ascontiguousarray(np.asarray(movie_table, dtype=np.float32))

    plans, counts, cnt_max, cap = _plan(users, movies)
    key = (tuple(cap.ravel()), tuple(cnt_max.ravel()))
    if _CACHE.get("key") != key:
        _CACHE["nc"], _CACHE["C"], _CACHE["L16"] = _build_nc(cap, cnt_max)
        _CACHE["key"] = key
    nc, C, L16 = _CACHE["nc"], _CACHE["C"], _CACHE["L16"]

    ncols = cap // 128
    colbase = {}
    cb = 0
    for i in range(UW_PER_CORE):
        for j in range(N_MW):
            colbase[(i, j)] = cb
            cb += int(ncols[i, j])

    in_maps = []
    slot_maps = []
    for c in range(N_CORES):
        pl = plans[c]
        elems, uwl, mwl = pl["elems"], pl["uwl"], pl["mwl"]

        base = c * SHARD_ROWS
        end = min(N_USERS, base + SHARD_ROWS)
        shard = np.zeros((SHARD_ROWS, EMB), dtype=np.float32)
        shard[: end - base] = user_table[base:end]

        u_flat = []
        m_flat = []
        batch_ids = []
        slots = []
        for i in range(UW_PER_CORE):
            for j in range(N_MW):
                cell = elems[(uwl == i) & (mwl == j)]
                cnt = cell.shape[0]
                l16 = L16[i][j]
                ul = np.zeros(l16, dtype=np.int16)
                ul[:cnt] = (users[cell] - (c * UW_PER_CORE + i) * W).astype(np.int16)
                u_flat.append(ul)
                ml = np.zeros(l16, dtype=np.int16)
                ml[:cnt] = (movies[cell] - j * W).astype(np.int16)
                m_flat.append(ml)
                s0 = colbase[(i, j)] * 128
                slots.append(s0 + np.arange(cnt))
                batch_ids.append(cell)
        u_flat = np.concatenate(u_flat)
        m_flat = np.concatenate(m_flat)
        slot_maps.append((np.concatenate(batch_ids), np.concatenate(slots)))

        in_maps.append(
            {
                "user_shard": shard,
                "movie_table": movie_table,
                "u_idx": np.ascontiguousarray(_wrap16(u_flat)),
                "m_idx": np.ascontiguousarray(_wrap16(m_flat)),
            }
        )

    trace = os.environ.get("KERNEL_TRACE", "") not in ("", "0")
    if trace:
        try:
            _install_ntff_hook()
        except Exception:
            trace = False
    res = run_bass_kernel_spmd(nc, in_maps, core_ids=list(range(N_CORES)), trace=trace)
    if trace:
        kernel.last_exec_time_ns = res.exec_time_ns
        kernel.last_trace = res.instructions_and_trace

    out = np.zeros((BATCH,), dtype=np.float32)
    for c in range(N_CORES):
        r = res.results[c]["out"]
        batch_ids, slots = slot_maps[c]
        out[batch_ids] = r[slots % 128, slots // 128]
    return out.reshape(BATCH, 1)
